# revision 2
# baseline (speedup 1.0000x reference)
"""JointFluxSingleTransformerBlockControl — TRN2 Bass kernel, 8-core tensor parallel.

Sharding (per core c of 8):
  - heads: 3 of 24  (q/k/v column-parallel, both streams)
  - mlp hidden: 1536 of 12288 rows
  - ada-norm emb rows: 1152 of 9216 (matvec sharded, device AllGather)
  - out-proj: column-parallel over this core's 1920 h-columns -> partial
    [3072, 1024] (T-layout) per stream; gate, out_b/8 and residual/8 are
    folded in on device so the host does a pure sum over cores.

Layout: activations in T-layout [feature=partition, seq=free]; weights are
pre-transposed on host so no on-device transposes are needed anywhere.

All matmul operands are bf16 (4x tensor-engine throughput vs fp32, half the
weight DMA); accumulation stays fp32 in PSUM. Stats rows (layernorm mu/var,
rms, softmax denominators) are computed and broadcast in fp32.
"""

import numpy as np
import ml_dtypes

import concourse.bass as bass
import concourse.bacc as bacc
import concourse.tile as tile
from concourse import mybir
from concourse.bass_utils import run_bass_kernel_spmd

F32 = mybir.dt.float32
BF16 = mybir.dt.bfloat16
AF = mybir.ActivationFunctionType

D = 3072
S = 1024
HD = 128
NCORES = 8
HPC = 3                  # heads per core
QO = HPC * HD            # 384 q/k/v out-dims per core
MLPC = 12288 // NCORES   # 1536
ES = 9216 // NCORES      # 1152 e-rows per core
KC = D // 128            # 24 contraction chunks
EPS = 1e-6
INV_SQRT_HD = float(1.0 / np.sqrt(128.0))
HKC = (QO + MLPC) // 128  # 15 h-col chunks per core


def bcast(ap, p=128):
    """Partition-broadcast a free-dims-only AP to [p, *free]."""
    return bass.AP(tensor=ap.tensor, offset=ap.offset, ap=[[0, p]] + list(ap.ap))


def build_nc():
    nc = bacc.Bacc(None, target_bir_lowering=False)
    dp = nc.declare_dram_parameter
    I = {}
    for s in ("m", "c"):
        I[f"xT_{s}"] = dp(f"xT_{s}", [D, S], BF16, isOutput=False)
        I[f"temb_{s}"] = dp(f"temb_{s}", [128, KC], F32, isOutput=False)
        I[f"normT_{s}"] = dp(f"normT_{s}", [D, ES], BF16, isOutput=False)
        I[f"nb_{s}"] = dp(f"nb_{s}", [1, ES], F32, isOutput=False)
        for w in ("q", "k", "v"):
            I[f"w{w}T_{s}"] = dp(f"w{w}T_{s}", [D, QO], BF16, isOutput=False)
        I[f"qb_{s}"] = dp(f"qb_{s}", [128, HPC], F32, isOutput=False)
        I[f"kb_{s}"] = dp(f"kb_{s}", [128, HPC], F32, isOutput=False)
        I[f"vb_{s}"] = dp(f"vb_{s}", [1, QO], F32, isOutput=False)
        I[f"rmsq_{s}"] = dp(f"rmsq_{s}", [128, 1], F32, isOutput=False)
        I[f"rmsk_{s}"] = dp(f"rmsk_{s}", [128, 1], F32, isOutput=False)
    I["mlpT"] = dp("mlpT", [D, MLPC], BF16, isOutput=False)
    I["mlpb"] = dp("mlpb", [128, MLPC // 128], F32, isOutput=False)
    I["outT"] = dp("outT", [HKC * 128, D], BF16, isOutput=False)
    I["outb"] = dp("outb", [128, KC], F32, isOutput=False)  # chunk-col layout
    I["cosT"] = dp("cosT", [128, S], F32, isOutput=False)
    I["sinT"] = dp("sinT", [128, S], F32, isOutput=False)
    I["rotT"] = dp("rotT", [128, 128], BF16, isOutput=False)
    OUT = {
        "m": dp("out_m", [D, S], BF16, isOutput=True),
        "c": dp("out_c", [D, S], BF16, isOutput=True),
    }

    with tile.TileContext(nc) as tc, nc.allow_low_precision("bf16 kernel, 2e-2 gate"):
        with (
            tc.tile_pool(name="dram", bufs=1, space="DRAM") as dram,
            tc.tile_pool(name="const", bufs=1) as const,
            tc.tile_pool(name="psum", bufs=8, space="PSUM") as psum,
            tc.tile_pool(name="rows", bufs=1) as rows,
        ):
            ones = const.tile([128, 1], BF16)
            nc.vector.memset(ones, 1.0)
            epst = const.tile([128, 1], F32)
            nc.vector.memset(epst, EPS)
            rotT = const.tile([128, 128], BF16, tag="rotT")
            nc.sync.dma_start(rotT, I["rotT"][:])
            cosT = const.tile([128, S], F32, tag="cosT")
            sinT = const.tile([128, S], F32, tag="sinT")
            nc.sync.dma_start(cosT, I["cosT"][:])
            nc.sync.dma_start(sinT, I["sinT"][:])
            mbt = const.tile([128, MLPC // 128], F32, tag="mlpb")
            nc.sync.dma_start(mbt, I["mlpb"][:])
            qkb = {}
            rwt = {}
            vbb = {}
            for s in ("m", "c"):
                for pj in ("q", "k"):
                    t = const.tile([128, HPC], F32, tag=f"{pj}b_{s}")
                    nc.sync.dma_start(t, I[f"{pj}b_{s}"][:])
                    qkb[(pj, s)] = t
                    r = const.tile([128, 1], F32, tag=f"rw_{pj}_{s}")
                    nc.sync.dma_start(r, I[f"rms{pj}_{s}"][:])
                    rwt[(pj, s)] = r
                v = const.tile([128, QO], F32, tag=f"vb_{s}")
                nc.sync.dma_start(v, bcast(I[f"vb_{s}"][0, :]))
                vbb[s] = v

            # ---------------- phase E: ada-norm matvec + AllGather --------
            e_bounce = dram.tile([2, ES], F32)
            ag_out = dram.tile([2 * NCORES, ES], F32)
            with tc.tile_pool(name="ph_e", bufs=2) as pe:
                for si, s in enumerate(("m", "c")):
                    st_f = pe.tile([128, KC], F32, tag="silu_f")
                    nc.sync.dma_start(st_f, I[f"temb_{s}"][:])
                    st_t = const.tile([128, KC], BF16, tag=f"silu_{s}")
                    nc.scalar.activation(st_t, st_f, AF.Silu)
                    eps_t = [psum.tile([1, 384], F32, tag="mm", name="eps_t") for _ in range(3)]
                    for kk in range(KC):
                        wn = pe.tile([128, ES], BF16, tag="wnorm")
                        nc.sync.dma_start(wn, I[f"normT_{s}"][kk * 128:(kk + 1) * 128, :])
                        for nt in range(3):
                            nc.tensor.matmul(
                                eps_t[nt], st_t[:, kk:kk + 1],
                                wn[:, nt * 384:(nt + 1) * 384],
                                start=(kk == 0), stop=(kk == KC - 1))
                    erow = pe.tile([1, ES], F32, tag="erow")
                    nbr = pe.tile([1, ES], F32, tag="nbrow")
                    nc.sync.dma_start(nbr, I[f"nb_{s}"][:])
                    for nt in range(3):
                        sl = slice(nt * 384, (nt + 1) * 384)
                        nc.vector.tensor_add(erow[:, sl], eps_t[nt], nbr[:, sl])
                    nc.sync.dma_start(e_bounce[si:si + 1, :], erow)
            nc.gpsimd.collective_compute(
                "AllGather", mybir.AluOpType.bypass,
                replica_groups=[list(range(NCORES))],
                ins=[e_bounce.opt()], outs=[ag_out.opt()])
            # ag_out row (2c+si) = core c stream si. Flat e index j*128+p with
            # chunk j = 9c+jj (ES = 9*128): view [two, p, c, jj].
            ag4 = ag_out[:].rearrange("(c two) (jj p) -> two p c jj", two=2, p=128)
            ss, scale1, g_sb, ob8 = {}, {}, {}, {}
            outb_cc = const.tile([128, KC], F32, tag="outb_cc")
            nc.sync.dma_start(outb_cc, I["outb"][:])
            for si, s in enumerate(("m", "c")):
                sst = const.tile([128, 48], F32, tag=f"ss_{s}")
                for cc in range(5):
                    nc.sync.dma_start(sst[:, cc * 9:(cc + 1) * 9],
                                      ag4[si, :, cc, :])
                nc.sync.dma_start(sst[:, 45:48], ag4[si, :, 5, 0:3])
                s1 = const.tile([128, KC], F32, tag=f"s1_{s}")
                nc.vector.tensor_scalar_add(s1, sst[:, 24:48], 1.0)
                ss[s], scale1[s] = sst, s1
                gt = const.tile([128, KC], F32, tag=f"gate_{s}")
                nc.sync.dma_start(gt[:, 0:6], ag4[si, :, 5, 3:9])
                for cc in (6, 7):
                    nc.sync.dma_start(gt[:, 6 + (cc - 6) * 9:6 + (cc - 5) * 9],
                                      ag4[si, :, cc, :])
                g_sb[s] = gt
                ot = const.tile([128, KC], F32, tag=f"ob8_{s}")
                nc.vector.tensor_mul(ot, gt, outb_cc)
                nc.vector.tensor_scalar_mul(ot, ot, 1.0 / NCORES)
                ob8[s] = ot

            rbounce = dram.tile([16, S], F32)
            rb_n = [0]

            def row_bcast(row_ap, dst_tile):
                i = rb_n[0] % 16
                rb_n[0] += 1
                nc.sync.dma_start(rbounce[i:i + 1, :], row_ap)
                nc.sync.dma_start(dst_tile, bcast(rbounce[i, :]))

            spq, spk, spv, spmlp = {}, {}, {}, {}
            with tc.tile_pool(name="nh", bufs=1) as nhp:
                for si, s in enumerate(("m", "c")):
                    # ---------- phase N: layernorm + ada scale/shift ------
                    nhT = nhp.tile([128, KC, S], BF16, tag="nhT")
                    with tc.tile_pool(name="ph_n", bufs=1) as pn:
                        sum_ps = [psum.tile([1, 512], F32, tag="mm", name="sum_ps") for _ in range(2)]
                        sq_ps = [psum.tile([1, 512], F32, tag="mm", name="sq_ps") for _ in range(2)]
                        for kk in range(KC):
                            xk = pn.tile([128, S], BF16, tag=f"xk{kk % 2}")
                            nc.sync.dma_start(xk, I[f"xT_{s}"][kk * 128:(kk + 1) * 128, :])
                            sq = pn.tile([128, S], BF16, tag=f"xsq{kk % 2}")
                            nc.vector.tensor_mul(sq, xk, xk)
                            for st in range(2):
                                sl = slice(st * 512, (st + 1) * 512)
                                nc.tensor.matmul(sum_ps[st], ones, xk[:, sl],
                                                 start=(kk == 0), stop=(kk == KC - 1))
                                nc.tensor.matmul(sq_ps[st], ones, sq[:, sl],
                                                 start=(kk == 0), stop=(kk == KC - 1))
                        mu = pn.tile([1, S], F32, tag="mu")
                        msq = pn.tile([1, S], F32, tag="msq")
                        for st in range(2):
                            sl = slice(st * 512, (st + 1) * 512)
                            nc.scalar.activation(mu[:, sl], sum_ps[st], AF.Copy,
                                                 scale=1.0 / D)
                            nc.scalar.activation(msq[:, sl], sq_ps[st], AF.Copy,
                                                 scale=1.0 / D)
                        var = pn.tile([1, S], F32, tag="var")
                        nc.vector.tensor_mul(var, mu, mu)
                        nc.vector.tensor_sub(var, msq, var)
                        rstd = pn.tile([1, S], F32, tag="rstd")
                        nc.scalar.activation(rstd, var, AF.Sqrt, bias=epst[:1, :])
                        nc.vector.reciprocal(rstd, rstd)
                        nmr = pn.tile([1, S], F32, tag="nmr")
                        nc.vector.tensor_mul(nmr, mu, rstd)
                        nc.vector.tensor_scalar_mul(nmr, nmr, -1.0)
                        rstd_bc = pn.tile([128, S], F32, tag="rstd_bc")
                        nmr_bc = pn.tile([128, S], F32, tag="nmr_bc")
                        row_bcast(rstd, rstd_bc)
                        row_bcast(nmr, nmr_bc)
                        for kk in range(KC):
                            xk = pn.tile([128, S], BF16, tag=f"xk{kk % 2}")
                            nc.sync.dma_start(xk, I[f"xT_{s}"][kk * 128:(kk + 1) * 128, :])
                            t1 = pn.tile([128, S], F32, tag=f"t1{kk % 2}")
                            nc.vector.tensor_mul(t1, xk, rstd_bc)
                            nc.vector.tensor_add(t1, t1, nmr_bc)
                            nc.scalar.activation(nhT[:, kk, :], t1, AF.Identity,
                                                 bias=ss[s][:, kk:kk + 1],
                                                 scale=scale1[s][:, kk:kk + 1])

                    # ---------- phase QKV ---------------------------------
                    with (
                        tc.tile_pool(name="ph_qkv1", bufs=1) as p1,
                        tc.tile_pool(name="ph_qkv2", bufs=2) as p2,
                        tc.tile_pool(name="ph_qkvw", bufs=3) as pw,
                    ):
                        for pj in ("q", "k"):
                            pps = [[psum.tile([128, 512], F32, tag="mm", name="pps")
                                    for _ in range(2)] for _ in range(HPC)]
                            for kk in range(KC):
                                wt = pw.tile([128, QO], BF16, tag="wqk")
                                nc.sync.dma_start(
                                    wt, I[f"w{pj}T_{s}"][kk * 128:(kk + 1) * 128, :])
                                for o in range(HPC):
                                    for st in range(2):
                                        nc.tensor.matmul(
                                            pps[o][st], wt[:, o * 128:(o + 1) * 128],
                                            nhT[:, kk, st * 512:(st + 1) * 512],
                                            start=(kk == 0), stop=(kk == KC - 1))
                            spill = dram.tile([QO, S], BF16, tag=f"sp_{pj}_{s}")
                            (spq if pj == "q" else spk)[s] = spill
                            for o in range(HPC):
                                raw = p2.tile([128, S], BF16, tag="raw")
                                for st in range(2):
                                    sl = slice(st * 512, (st + 1) * 512)
                                    nc.scalar.activation(
                                        raw[:, sl], pps[o][st], AF.Identity,
                                        bias=qkb[(pj, s)][:, o:o + 1])
                                sqh = p1.tile([128, S], BF16, tag="sqh")
                                nc.vector.tensor_mul(sqh, raw, raw)
                                rps = [psum.tile([1, 512], F32, tag="mm", name="rps")
                                       for _ in range(2)]
                                msr = p1.tile([1, S], F32, tag="msr")
                                for st in range(2):
                                    sl = slice(st * 512, (st + 1) * 512)
                                    nc.tensor.matmul(rps[st], ones, sqh[:, sl],
                                                     start=True, stop=True)
                                    nc.scalar.activation(msr[:, sl], rps[st],
                                                         AF.Copy, scale=1.0 / 128)
                                rsr = p1.tile([1, S], F32, tag="rsr")
                                nc.scalar.activation(rsr, msr, AF.Sqrt, bias=epst[:1, :])
                                nc.vector.reciprocal(rsr, rsr)
                                rs_bc = p1.tile([128, S], F32, tag="rs_bc")
                                row_bcast(rsr, rs_bc)
                                wq = p1.tile([128, S], BF16, tag="wq")
                                nc.scalar.activation(wq, raw, AF.Copy,
                                                     scale=rwt[(pj, s)])
                                rot_ps = [psum.tile([128, 512], F32, tag="mm", name="rot_ps")
                                          for _ in range(2)]
                                fin = p1.tile([128, S], F32, tag="fin")
                                for st in range(2):
                                    sl = slice(st * 512, (st + 1) * 512)
                                    nc.tensor.matmul(rot_ps[st], rotT, wq[:, sl],
                                                     start=True, stop=True)
                                    nc.vector.tensor_mul(fin[:, sl], rot_ps[st],
                                                         sinT[:, sl])
                                t2 = p1.tile([128, S], F32, tag="t2")
                                nc.vector.tensor_mul(t2, wq, cosT)
                                nc.vector.tensor_add(fin, fin, t2)
                                finb = p2.tile([128, S], BF16, tag="finb")
                                nc.vector.tensor_mul(finb, fin, rs_bc)
                                nc.sync.dma_start(spill[o * 128:(o + 1) * 128, :], finb)

                        # v projection (natural layout [seq, 384])
                        vps = [psum.tile([128, QO], F32, tag="mm", name="vps") for _ in range(8)]
                        for kk in range(KC):
                            wt = pw.tile([128, QO], BF16, tag="wqk")
                            nc.sync.dma_start(
                                wt, I[f"wvT_{s}"][kk * 128:(kk + 1) * 128, :])
                            for sc in range(8):
                                nc.tensor.matmul(
                                    vps[sc], nhT[:, kk, sc * 128:(sc + 1) * 128],
                                    wt, start=(kk == 0), stop=(kk == KC - 1))
                        vsp = dram.tile([S, QO], BF16, tag=f"sp_v_{s}")
                        spv[s] = vsp
                        for sc in range(8):
                            vt = p1.tile([128, QO], BF16, tag="vt")
                            nc.vector.tensor_add(vt, vps[sc], vbb[s])
                            nc.sync.dma_start(vsp[sc * 128:(sc + 1) * 128, :], vt)

                    # ---------- phase MLP ---------------------------------
                    msp = dram.tile([MLPC, S], BF16, tag=f"sp_mlp_{s}")
                    spmlp[s] = msp
                    with (
                        tc.tile_pool(name="ph_mlpw", bufs=3) as mw,
                        tc.tile_pool(name="ph_mlpo", bufs=2) as mo,
                    ):
                        for ob in range(3):
                            mps = [[psum.tile([128, 512], F32, tag="mm", name="mps")
                                    for _ in range(2)] for _ in range(4)]
                            for kk in range(KC):
                                wt = mw.tile([128, 512], BF16, tag="wmlp")
                                nc.sync.dma_start(
                                    wt, I["mlpT"][kk * 128:(kk + 1) * 128,
                                                  ob * 512:(ob + 1) * 512])
                                for o4 in range(4):
                                    for st in range(2):
                                        nc.tensor.matmul(
                                            mps[o4][st],
                                            wt[:, o4 * 128:(o4 + 1) * 128],
                                            nhT[:, kk, st * 512:(st + 1) * 512],
                                            start=(kk == 0), stop=(kk == KC - 1))
                            for o4 in range(4):
                                o = ob * 4 + o4
                                mt = mo.tile([128, S], BF16, tag="mt")
                                for st in range(2):
                                    sl = slice(st * 512, (st + 1) * 512)
                                    nc.scalar.activation(mt[:, sl], mps[o4][st],
                                                         AF.Gelu_apprx_tanh,
                                                         bias=mbt[:, o:o + 1])
                                nc.sync.dma_start(msp[o * 128:(o + 1) * 128, :], mt)

            # ---------------- phase ATTN ----------------------------------
            with tc.tile_pool(name="attn_out", bufs=1) as ao:
              with (
                tc.tile_pool(name="attn_qkv", bufs=1) as aq,
                tc.tile_pool(name="attn_wk", bufs=3) as awk,
                tc.tile_pool(name="attn_w1", bufs=2) as aw1,
              ):
                qm = aq.tile([128, HPC, S], BF16, tag="qm")
                am = ao.tile([128, HPC, S], BF16, tag="am")
                ac = ao.tile([128, HPC, S], BF16, tag="ac")
                att_out = {"m": am, "c": ac}
                for h in range(HPC):
                    nc.sync.dma_start(qm[:, h, :], spq["m"][h * 128:(h + 1) * 128, :])
                kt = vt = None
                for attn, (qs, ks, acc_t, fresh) in (
                    ("main", ("m", "m", am, True)),
                    ("ctrl", ("c", "c", ac, True)),
                    ("cross", ("m", "c", am, False)),
                ):
                    if attn != "cross":
                        kt = aq.tile([128, HPC, S], BF16, tag="kt")
                        vt = aq.tile([128, 8, QO], BF16, tag="vt")
                        for h in range(HPC):
                            nc.sync.dma_start(kt[:, h, :],
                                              spk[ks][h * 128:(h + 1) * 128, :])
                        for sc in range(8):
                            nc.sync.dma_start(vt[:, sc, :],
                                              spv[ks][sc * 128:(sc + 1) * 128, :])
                    if attn == "ctrl":
                        qt = aq.tile([128, HPC, S], BF16, tag="qc")
                        for h in range(HPC):
                            nc.sync.dma_start(qt[:, h, :],
                                              spq["c"][h * 128:(h + 1) * 128, :])
                    else:
                        qt = qm
                    for h in range(HPC):
                        av_ps = [psum.tile([128, 512], F32, tag="mm", name="av_ps") for _ in range(2)]
                        d_ps = [psum.tile([1, 512], F32, tag="mm", name="d_ps") for _ in range(2)]
                        for kv in range(8):
                            s_ps = [psum.tile([128, 512], F32, tag="mm", name="s_ps")
                                    for _ in range(2)]
                            et = awk.tile([128, S], BF16, tag="exp")
                            for st in range(2):
                                sl = slice(st * 512, (st + 1) * 512)
                                nc.tensor.matmul(s_ps[st],
                                                 kt[:, h, kv * 128:(kv + 1) * 128],
                                                 qt[:, h, sl], start=True, stop=True)
                                nc.scalar.activation(et[:, sl], s_ps[st], AF.Exp,
                                                     scale=INV_SQRT_HD)
                                nc.tensor.matmul(av_ps[st],
                                                 vt[:, kv, h * 128:(h + 1) * 128],
                                                 et[:, sl], start=(kv == 0),
                                                 stop=(kv == 7))
                                nc.tensor.matmul(d_ps[st], ones, et[:, sl],
                                                 start=(kv == 0), stop=(kv == 7))
                        den = aw1.tile([1, S], F32, tag="den")
                        for st in range(2):
                            nc.vector.reciprocal(den[:, st * 512:(st + 1) * 512],
                                                 d_ps[st])
                        den_bc = aw1.tile([128, S], F32, tag="den_bc")
                        row_bcast(den, den_bc)
                        for st in range(2):
                            sl = slice(st * 512, (st + 1) * 512)
                            if fresh:
                                nc.vector.tensor_mul(acc_t[:, h, sl], av_ps[st],
                                                     den_bc[:, sl])
                            else:
                                cr = aw1.tile([128, 512], BF16, tag="crs")
                                nc.vector.tensor_mul(cr, av_ps[st], den_bc[:, sl])
                                nc.vector.tensor_add(acc_t[:, h, sl],
                                                     acc_t[:, h, sl], cr)

              # ---------------- phase OUT-PROJ --------------------------
              with (
                  tc.tile_pool(name="hmlp", bufs=1) as hm,
                  tc.tile_pool(name="ow", bufs=2) as ow,
                  tc.tile_pool(name="fin", bufs=4) as fp,
              ):
                  mlp_sb = {}
                  for s in ("m", "c"):
                      mt = hm.tile([128, MLPC // 128, S], BF16, tag=f"hmlp_{s}")
                      for e in range(MLPC // 128):
                          nc.sync.dma_start(mt[:, e, :],
                                            spmlp[s][e * 128:(e + 1) * 128, :])
                      mlp_sb[s] = mt
                  for oc in range(KC):
                      warena = ow.tile([128, HKC, 128], BF16, tag="warena")
                      for kk in range(HKC):
                          nc.sync.dma_start(
                              warena[:, kk, :],
                              I["outT"][kk * 128:(kk + 1) * 128,
                                        oc * 128:(oc + 1) * 128])
                      for s in ("m", "c"):
                          ops_t = [psum.tile([128, 512], F32, tag="mm", name="ops_t")
                                   for _ in range(2)]
                          for kk in range(HKC):
                              rh = (att_out[s][:, kk, :] if kk < HPC
                                    else mlp_sb[s][:, kk - HPC, :])
                              for st in range(2):
                                  nc.tensor.matmul(
                                      ops_t[st], warena[:, kk, :],
                                      rh[:, st * 512:(st + 1) * 512],
                                      start=(kk == 0), stop=(kk == HKC - 1))
                          for st in range(2):
                              sl = slice(st * 512, (st + 1) * 512)
                              tg = fp.tile([128, 512], BF16, tag="tg")
                              nc.scalar.activation(tg, ops_t[st], AF.Identity,
                                                   scale=g_sb[s][:, oc:oc + 1],
                                                   bias=ob8[s][:, oc:oc + 1])
                              xs = fp.tile([128, 512], BF16, tag="xs")
                              nc.sync.dma_start(
                                  xs, I[f"xT_{s}"][oc * 128:(oc + 1) * 128, sl])
                              nc.vector.tensor_scalar_mul(xs, xs, 1.0 / NCORES)
                              nc.vector.tensor_add(tg, tg, xs)
                              nc.sync.dma_start(
                                  OUT[s][oc * 128:(oc + 1) * 128, sl], tg)
    nc.compile()
    return nc


_NC_CACHE = []


def _get_nc():
    if not _NC_CACHE:
        _NC_CACHE.append(build_nc())
    return _NC_CACHE[0]


def _prep_core_inputs(inputs, c):
    f = np.float32
    bf = ml_dtypes.bfloat16
    T = lambda a: np.ascontiguousarray(np.asarray(a, f).T)
    Tb = lambda a: np.ascontiguousarray(np.asarray(a, f).T.astype(bf))
    C = lambda a: np.ascontiguousarray(np.asarray(a, f))
    hs = np.asarray(inputs["hidden_states"], f)[0]
    hc = np.asarray(inputs["hidden_states_control"], f)[0]
    m = {}
    for s, x, te, nw, nb in (
        ("m", hs, inputs["temb"], inputs["norm_w"], inputs["norm_b"]),
        ("c", hc, inputs["temb_control"], inputs["normc_w"], inputs["normc_b"]),
    ):
        m[f"xT_{s}"] = Tb(x)
        m[f"temb_{s}"] = C(np.asarray(te, f)[0].reshape(KC, 128).T)
        m[f"normT_{s}"] = Tb(np.asarray(nw, f)[c * ES:(c + 1) * ES, :])
        m[f"nb_{s}"] = C(np.asarray(nb, f)[c * ES:(c + 1) * ES].reshape(1, ES))
    for s, pre in (("m", ""), ("c", "c")):
        for w in ("q", "k", "v"):
            W = np.asarray(inputs[f"{w}{pre}_w"], f)
            m[f"w{w}T_{s}"] = Tb(W[c * QO:(c + 1) * QO, :])
        for w in ("q", "k"):
            b = np.asarray(inputs[f"{w}{pre}_b"], f)[c * QO:(c + 1) * QO]
            m[f"{w}b_{s}"] = C(b.reshape(HPC, 128).T)
        m[f"vb_{s}"] = C(np.asarray(inputs[f"v{pre}_b"], f)[c * QO:(c + 1) * QO]
                         .reshape(1, QO))
        m[f"rmsq_{s}"] = C(np.asarray(inputs["rms_q" + pre], f).reshape(128, 1))
        m[f"rmsk_{s}"] = C(np.asarray(inputs["rms_k" + pre], f).reshape(128, 1))
    m["mlpT"] = Tb(np.asarray(inputs["mlp_w"], f)[c * MLPC:(c + 1) * MLPC, :])
    m["mlpb"] = C(np.asarray(inputs["mlp_b"], f)[c * MLPC:(c + 1) * MLPC]
                  .reshape(MLPC // 128, 128).T)
    ow = np.asarray(inputs["out_w"], f)
    m["outT"] = np.ascontiguousarray(np.concatenate(
        [ow[:, c * QO:(c + 1) * QO], ow[:, D + c * MLPC:D + (c + 1) * MLPC]],
        axis=1).T.astype(bf))
    m["outb"] = C(np.asarray(inputs["out_b"], f).reshape(KC, 128).T)
    m["cosT"] = T(inputs["rope_cos"])
    m["sinT"] = T(inputs["rope_sin"])
    R = np.zeros((128, 128), f)
    for i in range(64):
        R[2 * i, 2 * i + 1] = -1.0
        R[2 * i + 1, 2 * i] = 1.0
    m["rotT"] = np.ascontiguousarray(R.T.astype(bf))
    return m


def run_cores(inputs, trace=False):
    nc = _get_nc()
    in_maps = [_prep_core_inputs(inputs, c) for c in range(NCORES)]
    res = run_bass_kernel_spmd(nc, in_maps, list(range(NCORES)), trace=trace)
    h = np.sum([np.asarray(r["out_m"], np.float64) for r in res.results], axis=0)
    hc = np.sum([np.asarray(r["out_c"], np.float64) for r in res.results], axis=0)
    h = np.ascontiguousarray(h.T.astype(np.float32)).reshape(1, S, D)
    hc = np.ascontiguousarray(hc.T.astype(np.float32)).reshape(1, S, D)
    return (h, hc), res


def kernel(**inputs):
    out, _ = run_cores(inputs, trace=False)
    return out


# revision 12
# speedup vs baseline: 1.0576x; 1.0576x over previous
"""JointFluxSingleTransformerBlockControl — TRN2 Bass kernel, 8-core tensor parallel.

Sharding (per core c of 8):
  - heads: 3 of 24  (q/k/v column-parallel, both streams)
  - mlp hidden: 1536 of 12288 rows
  - ada-norm emb rows: 1152 of 9216 (matvec sharded, per-stream AllGather)
  - out-proj: column-parallel over this core's 1920 h-columns -> partial
    [3072, 1024] (T-layout) per stream; gate, out_b/8 and residual/8 are
    folded in on device so the host does a pure sum over cores.

Layout: activations in T-layout [feature=partition, seq=free]; weights are
pre-transposed on host so no on-device transposes are needed anywhere.

All matmul operands are bf16 (4x tensor-engine throughput vs fp32, half the
weight DMA); accumulation stays fp32 in PSUM. Row-broadcasts (rstd, rms,
softmax denominators) go through a K=1 ones-matmul instead of a DRAM bounce.
The layernorm apply is interleaved with the q-projection so the tensor
engine is not starved behind the DVE; both streams' stats run before the
first apply to cover the AllGather latency. Phase order groups same-
activation-table work (sqrt | gelu | exp) to minimize table reloads.
"""

import numpy as np
import ml_dtypes

import concourse.bass as bass
import concourse.bacc as bacc
import concourse.tile as tile
from concourse import mybir
from concourse.bass_utils import run_bass_kernel_spmd

F32 = mybir.dt.float32
BF16 = mybir.dt.bfloat16
AF = mybir.ActivationFunctionType

D = 3072
S = 1024
HD = 128
NCORES = 8
HPC = 3                  # heads per core
QO = HPC * HD            # 384 q/k/v out-dims per core
MLPC = 12288 // NCORES   # 1536
ES = 9216 // NCORES      # 1152 e-rows per core
KC = D // 128            # 24 contraction chunks
EPS = 1e-6
INV_SQRT_HD = float(1.0 / np.sqrt(128.0))
HKC = (QO + MLPC) // 128  # 15 h-col chunks per core


def bcast(ap, p=128):
    """Partition-broadcast a free-dims-only AP to [p, *free]."""
    return bass.AP(tensor=ap.tensor, offset=ap.offset, ap=[[0, p]] + list(ap.ap))


def build_nc():
    nc = bacc.Bacc(None, target_bir_lowering=False)
    dp = nc.declare_dram_parameter
    I = {}
    for s in ("m", "c"):
        I[f"xT_{s}"] = dp(f"xT_{s}", [D, S], BF16, isOutput=False)
        I[f"temb_{s}"] = dp(f"temb_{s}", [128, KC], F32, isOutput=False)
        I[f"normT_{s}"] = dp(f"normT_{s}", [D, ES], BF16, isOutput=False)
        I[f"nb_{s}"] = dp(f"nb_{s}", [1, ES], F32, isOutput=False)
        for w in ("q", "k", "v"):
            I[f"w{w}T_{s}"] = dp(f"w{w}T_{s}", [D, QO], BF16, isOutput=False)
        I[f"qb_{s}"] = dp(f"qb_{s}", [128, HPC], F32, isOutput=False)
        I[f"kb_{s}"] = dp(f"kb_{s}", [128, HPC], F32, isOutput=False)
        I[f"vb_{s}"] = dp(f"vb_{s}", [1, QO], F32, isOutput=False)
        I[f"rmsq_{s}"] = dp(f"rmsq_{s}", [128, 1], F32, isOutput=False)
        I[f"rmsk_{s}"] = dp(f"rmsk_{s}", [128, 1], F32, isOutput=False)
    I["mlpT"] = dp("mlpT", [D, MLPC], BF16, isOutput=False)
    I["mlpb"] = dp("mlpb", [128, MLPC // 128], F32, isOutput=False)
    # outT blocked per output chunk: row (oc*128 + p) holds the 15*128
    # h-contraction line for output dim block oc, partition p -> one
    # contiguous DMA per oc.
    I["outT"] = dp("outT", [KC * 128, HKC * 128], BF16, isOutput=False)
    I["outb"] = dp("outb", [128, KC], F32, isOutput=False)  # chunk-col layout
    I["cosT"] = dp("cosT", [128, S], BF16, isOutput=False)
    I["sinT"] = dp("sinT", [128, S], BF16, isOutput=False)
    I["rotT"] = dp("rotT", [128, 128], BF16, isOutput=False)
    OUT = {
        "m": dp("out_m", [D, S], BF16, isOutput=True),
        "c": dp("out_c", [D, S], BF16, isOutput=True),
    }

    with tile.TileContext(nc) as tc, nc.allow_low_precision("bf16 kernel, 2e-2 gate"):
        with (
            tc.tile_pool(name="dram", bufs=1, space="DRAM") as dram,
            tc.tile_pool(name="const", bufs=1) as const,
            tc.tile_pool(name="psum", bufs=8, space="PSUM") as psum,
        ):
            ones = const.tile([128, 1], BF16)
            nc.vector.memset(ones, 1.0)
            onesrow = const.tile([1, 128], BF16)
            nc.vector.memset(onesrow, 1.0)
            negrow = const.tile([1, 128], BF16)
            nc.vector.memset(negrow, -1.0)
            epst = const.tile([128, 1], F32)
            nc.vector.memset(epst, EPS)

            def mm_bcast(row, dst, lhs=None):
                """dst[128, S] (SBUF bf16) = broadcast of row[1, S] (bf16)
                via a K=1 matmul; ACT evacuates PSUM immediately."""
                if lhs is None:
                    lhs = onesrow
                for st in range(2):
                    sl = slice(st * 512, (st + 1) * 512)
                    bp = psum.tile([128, 512], F32, tag="mm", name="bc_ps")
                    nc.tensor.matmul(bp, lhs, row[:, sl], start=True, stop=True)
                    nc.scalar.activation(dst[:, sl], bp, AF.Copy)

            # ---------------- phase E: ada-norm matvec + AllGather --------
            # Per-stream AllGather so stream m's scale/shift is available as
            # early as possible (covers collective latency with stats work).
            ag = {}
            with tc.tile_pool(name="ph_e", bufs=2) as pe:
                for si, s in enumerate(("m", "c")):
                    st_f = pe.tile([128, KC], F32, tag="silu_f")
                    nc.sync.dma_start(st_f, I[f"temb_{s}"][:])
                    st_t = const.tile([128, KC], BF16, tag=f"silu_{s}")
                    nc.scalar.activation(st_t, st_f, AF.Silu)
                    eps_t = [psum.tile([1, 384], F32, tag="mm", name="eps_t") for _ in range(3)]
                    for kk in range(KC):
                        wn = pe.tile([128, ES], BF16, tag="wnorm")
                        nc.sync.dma_start(wn, I[f"normT_{s}"][kk * 128:(kk + 1) * 128, :])
                        for nt in range(3):
                            nc.tensor.matmul(
                                eps_t[nt], st_t[:, kk:kk + 1],
                                wn[:, nt * 384:(nt + 1) * 384],
                                start=(kk == 0), stop=(kk == KC - 1))
                    erow = pe.tile([1, ES], F32, tag="erow")
                    nbr = pe.tile([1, ES], F32, tag="nbrow")
                    nc.sync.dma_start(nbr, I[f"nb_{s}"][:])
                    for nt in range(3):
                        sl = slice(nt * 384, (nt + 1) * 384)
                        nc.vector.tensor_add(erow[:, sl], eps_t[nt], nbr[:, sl])
                    eb = dram.tile([1, ES], F32, tag=f"eb_{s}", name="eb")
                    nc.sync.dma_start(eb, erow)
                    ago = dram.tile([NCORES, ES], F32, tag=f"ag_{s}", name="ago")
                    nc.gpsimd.collective_compute(
                        "AllGather", mybir.AluOpType.bypass,
                        replica_groups=[list(range(NCORES))],
                        ins=[eb.opt()], outs=[ago.opt()])
                    ag[s] = ago

            # deferred const DMAs (not needed until QKV/MLP/out-proj)
            rotT = const.tile([128, 128], BF16, tag="rotT")
            nc.sync.dma_start(rotT, I["rotT"][:])
            cosT = const.tile([128, S], BF16, tag="cosT")
            sinT = const.tile([128, S], BF16, tag="sinT")
            nc.sync.dma_start(cosT, I["cosT"][:])
            nc.sync.dma_start(sinT, I["sinT"][:])
            mbt = const.tile([128, MLPC // 128], F32, tag="mlpb")
            nc.sync.dma_start(mbt, I["mlpb"][:])
            qkb = {}
            rwt = {}
            vbb = {}
            for s in ("m", "c"):
                for pj in ("q", "k"):
                    t = const.tile([128, HPC], F32, tag=f"{pj}b_{s}")
                    nc.sync.dma_start(t, I[f"{pj}b_{s}"][:])
                    qkb[(pj, s)] = t
                    r = const.tile([128, 1], F32, tag=f"rw_{pj}_{s}")
                    nc.sync.dma_start(r, I[f"rms{pj}_{s}"][:])
                    rwt[(pj, s)] = r
                v = const.tile([128, QO], F32, tag=f"vb_{s}")
                nc.sync.dma_start(v, bcast(I[f"vb_{s}"][0, :]))
                vbb[s] = v

            # scale/shift/gate unpack per stream.  ago row c = core c's
            # e-slice; flat e index c*ES + jj*128 + p: view [p, c, jj].
            ss, scale1, g_sb, ob8 = {}, {}, {}, {}
            outb_cc = const.tile([128, KC], F32, tag="outb_cc")
            nc.sync.dma_start(outb_cc, I["outb"][:])
            for si, s in enumerate(("m", "c")):
                ag3 = ag[s][:].rearrange("c (jj p) -> p c jj", p=128)
                sst = const.tile([128, 48], F32, tag=f"ss_{s}")
                for cc in range(5):
                    nc.sync.dma_start(sst[:, cc * 9:(cc + 1) * 9],
                                      ag3[:, cc, :])
                nc.sync.dma_start(sst[:, 45:48], ag3[:, 5, 0:3])
                s1 = const.tile([128, KC], F32, tag=f"s1_{s}")
                nc.vector.tensor_scalar_add(s1, sst[:, 24:48], 1.0)
                ss[s], scale1[s] = sst, s1
                gt = const.tile([128, KC], F32, tag=f"gate_{s}")
                nc.sync.dma_start(gt[:, 0:6], ag3[:, 5, 3:9])
                for cc in (6, 7):
                    nc.sync.dma_start(gt[:, 6 + (cc - 6) * 9:6 + (cc - 5) * 9],
                                      ag3[:, cc, :])
                g_sb[s] = gt
                ot = const.tile([128, KC], F32, tag=f"ob8_{s}")
                nc.vector.tensor_mul(ot, gt, outb_cc)
                nc.vector.tensor_scalar_mul(ot, ot, 1.0 / NCORES)
                ob8[s] = ot

            spq, spk, spv, spmlp = {}, {}, {}, {}
            nhT, rstd_bc, nmr_bc = {}, {}, {}
            with tc.tile_pool(name="nh", bufs=1) as nhp:
                for s in ("m", "c"):
                    nhT[s] = nhp.tile([128, KC, S], BF16, tag=f"nhT_{s}",
                                      name=f"nhT_{s}")
                    rstd_bc[s] = nhp.tile([128, S], BF16, tag=f"rstd_bc_{s}",
                                          name=f"rstd_bc_{s}")
                    nmr_bc[s] = nhp.tile([128, S], BF16, tag=f"nmr_bc_{s}",
                                         name=f"nmr_bc_{s}")
                with (
                    tc.tile_pool(name="ph_n", bufs=1) as pn,
                    tc.tile_pool(name="ph_qkv1", bufs=2) as p1,
                    tc.tile_pool(name="ph_qkv2", bufs=2) as p2,
                    tc.tile_pool(name="ph_qkvw", bufs=3) as pw,
                ):
                    # -- layernorm stats, both streams up front (covers the
                    #    AllGather latency with PE work) --
                    for s in ("m", "c"):
                        sum_ps = [psum.tile([1, 512], F32, tag="mm", name="sum_ps") for _ in range(2)]
                        sq_ps = [psum.tile([1, 512], F32, tag="mm", name="sq_ps") for _ in range(2)]
                        for kk in range(KC):
                            xk = pn.tile([128, S], BF16, tag=f"xk{kk % 3}")
                            nc.sync.dma_start(xk,
                                              I[f"xT_{s}"][kk * 128:(kk + 1) * 128, :])
                            sq = pn.tile([128, S], BF16, tag=f"xsq{kk % 2}")
                            nc.vector.tensor_mul(sq, xk, xk)
                            for st in range(2):
                                sl = slice(st * 512, (st + 1) * 512)
                                nc.tensor.matmul(sum_ps[st], ones, xk[:, sl],
                                                 start=(kk == 0), stop=(kk == KC - 1))
                                nc.tensor.matmul(sq_ps[st], ones, sq[:, sl],
                                                 start=(kk == 0), stop=(kk == KC - 1))
                        mu = pn.tile([1, S], F32, tag="mu")
                        msq = pn.tile([1, S], F32, tag="msq")
                        for st in range(2):
                            sl = slice(st * 512, (st + 1) * 512)
                            nc.scalar.activation(mu[:, sl], sum_ps[st], AF.Copy,
                                                 scale=1.0 / D)
                            nc.scalar.activation(msq[:, sl], sq_ps[st], AF.Copy,
                                                 scale=1.0 / D)
                        var = pn.tile([1, S], F32, tag="var")
                        nc.vector.tensor_mul(var, mu, mu)
                        nc.vector.tensor_sub(var, msq, var)
                        rstd = pn.tile([1, S], F32, tag="rstd")
                        nc.scalar.activation(rstd, var, AF.Sqrt, bias=epst[:1, :])
                        rstd_row = pn.tile([1, S], BF16, tag="rstd_row")
                        nc.vector.reciprocal(rstd_row, rstd)
                        nmr_row = pn.tile([1, S], BF16, tag="nmr_row")
                        nc.vector.tensor_mul(nmr_row, mu, rstd_row)
                        mm_bcast(rstd_row, rstd_bc[s])
                        mm_bcast(nmr_row, nmr_bc[s], lhs=negrow)  # -mu*rstd

                    def qk_head_post(pj, s, o, pps_o, spill):
                        """rms-norm + rope one head from its accum PSUM."""
                        raw = p2.tile([128, S], BF16, tag="raw")
                        for st in range(2):
                            sl = slice(st * 512, (st + 1) * 512)
                            nc.scalar.activation(
                                raw[:, sl], pps_o[st], AF.Identity,
                                bias=qkb[(pj, s)][:, o:o + 1])
                        sqh = p1.tile([128, S], BF16, tag="sqh")
                        nc.vector.tensor_mul(sqh, raw, raw)
                        rps = [psum.tile([1, 512], F32, tag="mm", name="rps")
                               for _ in range(2)]
                        rsr = p1.tile([1, S], F32, tag="rsr")
                        for st in range(2):
                            sl = slice(st * 512, (st + 1) * 512)
                            nc.tensor.matmul(rps[st], ones, sqh[:, sl],
                                             start=True, stop=True)
                            nc.scalar.activation(rsr[:, sl], rps[st], AF.Sqrt,
                                                 scale=1.0 / 128,
                                                 bias=epst[:1, :])
                        rsr_row = p1.tile([1, S], BF16, tag="rsr_row")
                        nc.vector.reciprocal(rsr_row, rsr)
                        rs_bc = p1.tile([128, S], BF16, tag="rs_bc")
                        mm_bcast(rsr_row, rs_bc)
                        wq = p1.tile([128, S], BF16, tag="wq")
                        nc.vector.tensor_scalar_mul(wq, raw, rwt[(pj, s)][:, 0:1])
                        fin = p1.tile([128, S], BF16, tag="fin")
                        for st in range(2):
                            sl = slice(st * 512, (st + 1) * 512)
                            rot_ps = psum.tile([128, 512], F32, tag="mm",
                                               name="rot_ps")
                            nc.tensor.matmul(rot_ps, rotT, wq[:, sl],
                                             start=True, stop=True)
                            nc.vector.tensor_mul(fin[:, sl], rot_ps,
                                                 sinT[:, sl])
                        t2 = p1.tile([128, S], BF16, tag="t2")
                        nc.vector.tensor_mul(t2, wq, cosT)
                        nc.vector.tensor_add(fin, fin, t2)
                        finb = p2.tile([128, S], BF16, tag="finb")
                        nc.vector.tensor_mul(finb, fin, rs_bc)
                        nc.sync.dma_start(spill[o * 128:(o + 1) * 128, :], finb)

                    for s in ("m", "c"):
                        # -- q proj interleaved with layernorm apply --
                        pps = [[psum.tile([128, 512], F32, tag="mm", name="pps")
                                for _ in range(2)] for _ in range(HPC)]
                        for kk in range(KC):
                            xk = pn.tile([128, S], BF16, tag=f"xk{kk % 3}")
                            nc.sync.dma_start(xk,
                                              I[f"xT_{s}"][kk * 128:(kk + 1) * 128, :])
                            t1 = pn.tile([128, S], BF16, tag=f"t1{kk % 2}")
                            nc.vector.tensor_mul(t1, xk, rstd_bc[s])
                            nc.vector.tensor_add(t1, t1, nmr_bc[s])
                            nc.scalar.activation(nhT[s][:, kk, :], t1, AF.Identity,
                                                 bias=ss[s][:, kk:kk + 1],
                                                 scale=scale1[s][:, kk:kk + 1])
                            wt = pw.tile([128, QO], BF16, tag="wqk")
                            nc.sync.dma_start(
                                wt, I[f"wqT_{s}"][kk * 128:(kk + 1) * 128, :])
                            for o in range(HPC):
                                for st in range(2):
                                    nc.tensor.matmul(
                                        pps[o][st], wt[:, o * 128:(o + 1) * 128],
                                        nhT[s][:, kk, st * 512:(st + 1) * 512],
                                        start=(kk == 0), stop=(kk == KC - 1))
                        spill = dram.tile([QO, S], BF16, tag=f"sp_q_{s}")
                        spq[s] = spill
                        for o in range(HPC):
                            qk_head_post("q", s, o, pps[o], spill)

                        # -- k proj --
                        pps = [[psum.tile([128, 512], F32, tag="mm", name="pps")
                                for _ in range(2)] for _ in range(HPC)]
                        for kk in range(KC):
                            wt = pw.tile([128, QO], BF16, tag="wqk")
                            nc.sync.dma_start(
                                wt, I[f"wkT_{s}"][kk * 128:(kk + 1) * 128, :])
                            for o in range(HPC):
                                for st in range(2):
                                    nc.tensor.matmul(
                                        pps[o][st], wt[:, o * 128:(o + 1) * 128],
                                        nhT[s][:, kk, st * 512:(st + 1) * 512],
                                        start=(kk == 0), stop=(kk == KC - 1))
                        spill = dram.tile([QO, S], BF16, tag=f"sp_k_{s}")
                        spk[s] = spill
                        for o in range(HPC):
                            qk_head_post("k", s, o, pps[o], spill)

                        # -- v projection (natural layout [seq, 384]) --
                        vps = [psum.tile([128, QO], F32, tag="mm", name="vps") for _ in range(8)]
                        for kk in range(KC):
                            wt = pw.tile([128, QO], BF16, tag="wqk")
                            nc.sync.dma_start(
                                wt, I[f"wvT_{s}"][kk * 128:(kk + 1) * 128, :])
                            for sc in range(8):
                                nc.tensor.matmul(
                                    vps[sc], nhT[s][:, kk, sc * 128:(sc + 1) * 128],
                                    wt, start=(kk == 0), stop=(kk == KC - 1))
                        vsp = dram.tile([S, QO], BF16, tag=f"sp_v_{s}")
                        spv[s] = vsp
                        for sc in range(8):
                            vt = p1.tile([128, QO], BF16, tag="vt")
                            nc.vector.tensor_add(vt, vps[sc], vbb[s])
                            nc.sync.dma_start(vsp[sc * 128:(sc + 1) * 128, :], vt)

                # ---------- phase MLP (gelu act table); weight arena is
                # loaded once per ob and shared by both streams -----------
                for s in ("m", "c"):
                    spmlp[s] = dram.tile([MLPC, S], BF16, tag=f"sp_mlp_{s}",
                                         name=f"sp_mlp_{s}")
                with (
                    tc.tile_pool(name="ph_mlpw", bufs=2) as mw,
                    tc.tile_pool(name="ph_mlpo", bufs=2) as mo,
                ):
                    for ob in range(3):
                        wa = mw.tile([128, KC, 512], BF16, tag="wmlp")
                        for kk in range(KC):
                            nc.sync.dma_start(
                                wa[:, kk, :],
                                I["mlpT"][kk * 128:(kk + 1) * 128,
                                          ob * 512:(ob + 1) * 512])
                        for s in ("m", "c"):
                            mps = [[psum.tile([128, 512], F32, tag="mm", name="mps")
                                    for _ in range(2)] for _ in range(4)]
                            for kk in range(KC):
                                for o4 in range(4):
                                    for st in range(2):
                                        nc.tensor.matmul(
                                            mps[o4][st],
                                            wa[:, kk, o4 * 128:(o4 + 1) * 128],
                                            nhT[s][:, kk, st * 512:(st + 1) * 512],
                                            start=(kk == 0), stop=(kk == KC - 1))
                            for o4 in range(4):
                                o = ob * 4 + o4
                                mt = mo.tile([128, S], BF16, tag="mt")
                                for st in range(2):
                                    sl = slice(st * 512, (st + 1) * 512)
                                    nc.scalar.activation(mt[:, sl], mps[o4][st],
                                                         AF.Gelu_apprx_tanh,
                                                         bias=mbt[:, o:o + 1])
                                nc.sync.dma_start(
                                    spmlp[s][o * 128:(o + 1) * 128, :], mt)

            # ---------------- phase ATTN ----------------------------------
            with tc.tile_pool(name="attn_out", bufs=1) as ao:
              with (
                tc.tile_pool(name="attn_qkv", bufs=1) as aq,
                tc.tile_pool(name="attn_wk", bufs=3) as awk,
                tc.tile_pool(name="attn_w1", bufs=2) as aw1,
              ):
                qm = aq.tile([128, HPC, S], BF16, tag="qm")
                am = ao.tile([128, HPC, S], BF16, tag="am")
                ac = ao.tile([128, HPC, S], BF16, tag="ac")
                att_out = {"m": am, "c": ac}
                for h in range(HPC):
                    nc.sync.dma_start(qm[:, h, :], spq["m"][h * 128:(h + 1) * 128, :])
                kt = vt = None
                for attn, (qs, ks, acc_t, fresh) in (
                    ("main", ("m", "m", am, True)),
                    ("ctrl", ("c", "c", ac, True)),
                    ("cross", ("m", "c", am, False)),
                ):
                    if attn != "cross":
                        kt = aq.tile([128, HPC, S], BF16, tag="kt")
                        vt = aq.tile([128, 8, QO], BF16, tag="vt")
                        for h in range(HPC):
                            nc.sync.dma_start(kt[:, h, :],
                                              spk[ks][h * 128:(h + 1) * 128, :])
                        for sc in range(8):
                            nc.sync.dma_start(vt[:, sc, :],
                                              spv[ks][sc * 128:(sc + 1) * 128, :])
                    if attn == "ctrl":
                        qt = aq.tile([128, HPC, S], BF16, tag="qc")
                        for h in range(HPC):
                            nc.sync.dma_start(qt[:, h, :],
                                              spq["c"][h * 128:(h + 1) * 128, :])
                    else:
                        qt = qm
                    for h in range(HPC):
                        av_ps = [psum.tile([128, 512], F32, tag="mm", name="av_ps") for _ in range(2)]
                        esum = aw1.tile([128, S], BF16, tag="esum")
                        for kv in range(8):
                            s_ps = [psum.tile([128, 512], F32, tag="mm", name="s_ps")
                                    for _ in range(2)]
                            et = awk.tile([128, S], BF16, tag="exp")
                            for st in range(2):
                                sl = slice(st * 512, (st + 1) * 512)
                                nc.tensor.matmul(s_ps[st],
                                                 kt[:, h, kv * 128:(kv + 1) * 128],
                                                 qt[:, h, sl], start=True, stop=True)
                                nc.scalar.activation(et[:, sl], s_ps[st], AF.Exp,
                                                     scale=INV_SQRT_HD)
                                nc.tensor.matmul(av_ps[st],
                                                 vt[:, kv, h * 128:(h + 1) * 128],
                                                 et[:, sl], start=(kv == 0),
                                                 stop=(kv == 7))
                                if kv == 0:
                                    nc.vector.tensor_scalar_add(
                                        esum[:, sl], et[:, sl], 0.0)
                                else:
                                    nc.vector.tensor_add(
                                        esum[:, sl], esum[:, sl], et[:, sl])
                        d_ps = [psum.tile([1, 512], F32, tag="mm", name="d_ps")
                                for _ in range(2)]
                        den_row = aw1.tile([1, S], BF16, tag="den")
                        for st in range(2):
                            sl = slice(st * 512, (st + 1) * 512)
                            nc.tensor.matmul(d_ps[st], ones, esum[:, sl],
                                             start=True, stop=True)
                            nc.vector.reciprocal(den_row[:, sl], d_ps[st])
                        den_bc = aw1.tile([128, S], BF16, tag="den_bc")
                        mm_bcast(den_row, den_bc)
                        for st in range(2):
                            sl = slice(st * 512, (st + 1) * 512)
                            if fresh:
                                nc.vector.tensor_mul(acc_t[:, h, sl], av_ps[st],
                                                     den_bc[:, sl])
                            else:
                                cr = aw1.tile([128, 512], BF16, tag="crs")
                                nc.vector.tensor_mul(cr, av_ps[st], den_bc[:, sl])
                                nc.vector.tensor_add(acc_t[:, h, sl],
                                                     acc_t[:, h, sl], cr)

              # ---------------- phase OUT-PROJ --------------------------
              with (
                  tc.tile_pool(name="hmlp", bufs=1) as hm,
                  tc.tile_pool(name="ow", bufs=2) as ow,
                  tc.tile_pool(name="fin", bufs=4) as fp,
              ):
                  mlp_sb = {}
                  for s in ("m", "c"):
                      mt = hm.tile([128, MLPC // 128, S], BF16, tag=f"hmlp_{s}")
                      for e in range(MLPC // 128):
                          nc.sync.dma_start(mt[:, e, :],
                                            spmlp[s][e * 128:(e + 1) * 128, :])
                      mlp_sb[s] = mt
                  for oc in range(KC):
                      warena = ow.tile([128, HKC, 128], BF16, tag="warena")
                      nc.sync.dma_start(
                          warena[:].rearrange("p kk c -> p (kk c)"),
                          I["outT"][oc * 128:(oc + 1) * 128, :])
                      for s in ("m", "c"):
                          ops_t = [psum.tile([128, 512], F32, tag="mm", name="ops_t")
                                   for _ in range(2)]
                          for kk in range(HKC):
                              rh = (att_out[s][:, kk, :] if kk < HPC
                                    else mlp_sb[s][:, kk - HPC, :])
                              for st in range(2):
                                  nc.tensor.matmul(
                                      ops_t[st], warena[:, kk, :],
                                      rh[:, st * 512:(st + 1) * 512],
                                      start=(kk == 0), stop=(kk == HKC - 1))
                          for st in range(2):
                              sl = slice(st * 512, (st + 1) * 512)
                              tg = fp.tile([128, 512], BF16, tag="tg")
                              nc.scalar.activation(tg, ops_t[st], AF.Identity,
                                                   scale=g_sb[s][:, oc:oc + 1],
                                                   bias=ob8[s][:, oc:oc + 1])
                              xs = fp.tile([128, 512], BF16, tag="xs")
                              nc.sync.dma_start(
                                  xs, I[f"xT_{s}"][oc * 128:(oc + 1) * 128, sl])
                              nc.vector.tensor_scalar_mul(xs, xs, 1.0 / NCORES)
                              nc.vector.tensor_add(tg, tg, xs)
                              nc.sync.dma_start(
                                  OUT[s][oc * 128:(oc + 1) * 128, sl], tg)
    nc.compile()
    return nc


_NC_CACHE = []


def _get_nc():
    if not _NC_CACHE:
        _NC_CACHE.append(build_nc())
    return _NC_CACHE[0]


def _prep_core_inputs(inputs, c):
    f = np.float32
    bf = ml_dtypes.bfloat16
    T = lambda a: np.ascontiguousarray(np.asarray(a, f).T)
    Tb = lambda a: np.ascontiguousarray(np.asarray(a, f).T.astype(bf))
    C = lambda a: np.ascontiguousarray(np.asarray(a, f))
    hs = np.asarray(inputs["hidden_states"], f)[0]
    hc = np.asarray(inputs["hidden_states_control"], f)[0]
    m = {}
    for s, x, te, nw, nb in (
        ("m", hs, inputs["temb"], inputs["norm_w"], inputs["norm_b"]),
        ("c", hc, inputs["temb_control"], inputs["normc_w"], inputs["normc_b"]),
    ):
        m[f"xT_{s}"] = Tb(x)
        m[f"temb_{s}"] = C(np.asarray(te, f)[0].reshape(KC, 128).T)
        m[f"normT_{s}"] = Tb(np.asarray(nw, f)[c * ES:(c + 1) * ES, :])
        m[f"nb_{s}"] = C(np.asarray(nb, f)[c * ES:(c + 1) * ES].reshape(1, ES))
    for s, pre in (("m", ""), ("c", "c")):
        for w in ("q", "k", "v"):
            W = np.asarray(inputs[f"{w}{pre}_w"], f)
            m[f"w{w}T_{s}"] = Tb(W[c * QO:(c + 1) * QO, :])
        for w in ("q", "k"):
            b = np.asarray(inputs[f"{w}{pre}_b"], f)[c * QO:(c + 1) * QO]
            m[f"{w}b_{s}"] = C(b.reshape(HPC, 128).T)
        m[f"vb_{s}"] = C(np.asarray(inputs[f"v{pre}_b"], f)[c * QO:(c + 1) * QO]
                         .reshape(1, QO))
        m[f"rmsq_{s}"] = C(np.asarray(inputs["rms_q" + pre], f).reshape(128, 1))
        m[f"rmsk_{s}"] = C(np.asarray(inputs["rms_k" + pre], f).reshape(128, 1))
    m["mlpT"] = Tb(np.asarray(inputs["mlp_w"], f)[c * MLPC:(c + 1) * MLPC, :])
    m["mlpb"] = C(np.asarray(inputs["mlp_b"], f)[c * MLPC:(c + 1) * MLPC]
                  .reshape(MLPC // 128, 128).T)
    ow = np.asarray(inputs["out_w"], f)
    W = np.concatenate(
        [ow[:, c * QO:(c + 1) * QO], ow[:, D + c * MLPC:D + (c + 1) * MLPC]],
        axis=1)  # [3072 d, 1920 h]
    # blocked: [oc, p, kk, col] = W[oc*128+col, kk*128+p]
    W4 = W.reshape(KC, 128, HKC, 128).transpose(0, 3, 2, 1)
    m["outT"] = np.ascontiguousarray(
        W4.reshape(KC * 128, HKC * 128).astype(bf))
    m["outb"] = C(np.asarray(inputs["out_b"], f).reshape(KC, 128).T)
    m["cosT"] = np.ascontiguousarray(np.asarray(inputs["rope_cos"], f).T.astype(bf))
    m["sinT"] = np.ascontiguousarray(np.asarray(inputs["rope_sin"], f).T.astype(bf))
    R = np.zeros((128, 128), f)
    for i in range(64):
        R[2 * i, 2 * i + 1] = -1.0
        R[2 * i + 1, 2 * i] = 1.0
    m["rotT"] = np.ascontiguousarray(R.T.astype(bf))
    return m


def run_cores(inputs, trace=False):
    nc = _get_nc()
    in_maps = [_prep_core_inputs(inputs, c) for c in range(NCORES)]
    res = run_bass_kernel_spmd(nc, in_maps, list(range(NCORES)), trace=trace)
    h = np.sum([np.asarray(r["out_m"], np.float64) for r in res.results], axis=0)
    hc = np.sum([np.asarray(r["out_c"], np.float64) for r in res.results], axis=0)
    h = np.ascontiguousarray(h.T.astype(np.float32)).reshape(1, S, D)
    hc = np.ascontiguousarray(hc.T.astype(np.float32)).reshape(1, S, D)
    return (h, hc), res


def kernel(**inputs):
    out, _ = run_cores(inputs, trace=False)
    return out
